# revision 68
# baseline (speedup 1.0000x reference)
"""Multi-head attention (B=8, S=1024, D=1024, H=16) on 8 TRN2 NeuronCores.

Sharding: pure data-parallel over batch - each core computes one batch
element end-to-end (weights replicated per core), so no collectives.

Per-core pipeline (PE-bound design: ~177us of matmul rows at bf16, with
softmax exp on ScalarE (~133us) hidden underneath):
  1. Inputs DMA'd with f32->bf16 cast (SWDGE), ordered on the serial DMA
     FIFO so pair-0's projections start ~15us in; weights load into
     persistent tiles in per-2-pair column slices. A burst of junk
     matmuls at t~1us walks the PE p-state up to full clock before real
     work arrives.
  2. PE-transposes pack inputs into [P, kc, S] (contraction on
     partitions) via a scoped 2-bank PSUM pool; pair-0's QK projection
     matmuls slip underneath the DMA-paced transposes through the
     engines' 4-deep wait-queue bypass.
  3. Per head-pair p: QT/KT[p] = (x @ w)^T in f32 (f32r matmuls, bias
     folded into the PSUM drain); V natural orientation into VA[p]
     [j_part, jb, head, 65] (bf16) with a ones column.
  4. 16 score/exp slots per pair (head x j-block): scores^T[j,i] via 2
     f32r matmuls into a [P, S] PSUM tile (2 banks, bufs=3 ring); one
     exp(s/8 - 2.4) per slot on ScalarE straight out of PSUM -> bf16
     eT[p] [j_part, jb, head, i].
  5. PV natural: out[i, dk+1] accumulated over jb (lhsT = eT block,
     rhs = VA block); col 64 = sum(exp) via the ones column. Reciprocal
     + per-partition-scalar normalize into a per-pair staging tile, one
     [P, ib, 128] DMA per pair (split per head at the tail).
  6. Emission is software-pipelined: projections for pair p+1, PV of
     pair p (h0) and of pair p-1 (h1) ride in pair p's slots so the
     in-order PE queue never head-of-line blocks.
     NOTE: a PSUM bank must only ever hold ONE open matmul accumulation
     group (start_tensor_calc zeroing is region-granular), so PV is
     strictly ib-major.
  7. End-game scheduling (the kernel's finish = Act-stream end + tail,
     and Act saturates ~25us before the end): fills are split into
     ~0.85us quanta (qk/v quarter units) and spread one per score slot;
     pair-6's stretch absorbs ALL of pair-7's h0 score slots plus
     (7,1,0..1) as run-ahead (eT ring bufs=2 allows exactly one pair of
     look-ahead); the final six scores issue with nothing in front of
     them and all deferrable PV/V work runs after them, overlapping the
     last exps via the framework's fine-grained per-matmul waits.
     Pair-0's Act-bound slots host the v transposes + pair-1's late
     projection quarters, pulling the first score slot ~7us earlier.
     Stretch-junction fills must not depend on the previous stretch's
     last exp (pv(p-1,h1) sits at slots 8/9, not 0/1).
     Baseline 199564ns -> 195071ns (TimelineSim; identical numerics).
"""
import numpy as np
from contextlib import ExitStack

import concourse.bass as bass
import concourse.mybir as mybir
import concourse.tile as tile
from concourse import bacc
from concourse.bass_utils import run_bass_kernel_spmd
from concourse.masks import make_identity

F32 = mybir.dt.float32
F32R = mybir.dt.float32r
BF16 = mybir.dt.bfloat16
FP8 = mybir.dt.float8e4
DR = mybir.MatmulPerfMode.DoubleRow
EXP = mybir.ActivationFunctionType.Exp

B, S, D, H, DK = 8, 1024, 1024, 16, 64
P = 128
NB = S // P            # 8 row/col blocks
NPAIR = H // 2         # 8 head pairs
HALF = 512
N_CORES = 8
EXP_SCALE = 0.125      # 1/sqrt(dk)
EXP_BIAS = -2.4        # keeps bf16 exp well-scaled; fp8 would need <240

PV_FP8 = False         # probs/V in fp8e4 + DoubleRow PV (else bf16).
                       # fp8 measured 6e-2 rel err: attention rows here are
                       # concentrated, so 3.6% fp8 noise on dominant probs
                       # does not average out. bf16 keeps 7e-3.
_NO_DR = False         # debug: fp8 without DoubleRow perf mode
TAIL_JB_MAJOR = False  # pipeline the final head's PV across exp slots

_compiled = {}


def _build(use_bias=True, pv_fp8=PV_FP8):
    nc = bacc.Bacc("TRN2", target_bir_lowering=False, debug=False,
                   enable_asserts=False, num_devices=N_CORES)

    dq = nc.dram_tensor("q", [S, D], F32, kind="ExternalInput").ap()
    dk_ = nc.dram_tensor("k", [S, D], F32, kind="ExternalInput").ap()
    dv = nc.dram_tensor("v", [S, D], F32, kind="ExternalInput").ap()
    dwq = nc.dram_tensor("wq", [D, D], F32, kind="ExternalInput").ap()
    dwk = nc.dram_tensor("wk", [D, D], F32, kind="ExternalInput").ap()
    dwv = nc.dram_tensor("wv", [D, D], F32, kind="ExternalInput").ap()
    dbq = nc.dram_tensor("bq", [D], F32, kind="ExternalInput").ap()
    dbk = nc.dram_tensor("bk", [D], F32, kind="ExternalInput").ap()
    dbv = nc.dram_tensor("bv", [D], F32, kind="ExternalInput").ap()
    dout = nc.dram_tensor("out", [S, D], F32, kind="ExternalOutput").ap()

    PVDT = FP8 if pv_fp8 else BF16

    with tile.TileContext(nc) as tc:
        with ExitStack() as ctx:
            const = ctx.enter_context(tc.tile_pool(name="const", bufs=1))
            persist = ctx.enter_context(tc.tile_pool(name="persist", bufs=1))
            ring = ctx.enter_context(tc.tile_pool(name="ring", bufs=1))
            scr = ctx.enter_context(tc.tile_pool(name="scr", bufs=1,
                                                 space="PSUM"))

            ident_bf = const.tile([P, P], BF16)
            ones_bf = const.tile([1, P], BF16)
            ebias = const.tile([P, 1], F32)
            junk = const.tile([P, DK], BF16)
            actwarm = const.tile([1, 1], F32)

            qt = persist.tile([P, NB, S], BF16, name="qt")
            kt = persist.tile([P, NB, S], BF16, name="kt")
            vt = persist.tile([P, NB, S], BF16, name="vt")
            wqa = persist.tile([P, NB, D], BF16, name="wqa")
            wka = persist.tile([P, NB, D], BF16, name="wka")
            wva = persist.tile([P, NB, D], BF16, name="wva")
            bqt = persist.tile([P, NPAIR], F32, name="bqt")
            bkt = persist.tile([P, NPAIR], F32, name="bkt")
            bvr = persist.tile([1, D], BF16, name="bvr")

            # ---- DMA FIFO order tuned so pair-0's QK projections can
            # start as early as possible on the serial DMA device ----
            def load_nat(dsrc, nm, chunks):
                tiles = []
                for ci in chunks:
                    rr = ci * 2
                    natt = ring.tile([P, 2, S], BF16, name=f"nat_{nm}_{rr}",
                                     tag="nat", bufs=4)
                    nc.gpsimd.dma_start(
                        out=natt[:],
                        in_=dsrc[rr * P:(rr + 2) * P, :]
                            .rearrange("(r p) d -> p r d", p=P))
                    tiles.append(natt)
                return tiles

            def load_w_slice(wdst, wsrc, m, span=1):
                nc.gpsimd.dma_start(
                    out=wdst[:, :, m * 2 * P:(m + span) * 2 * P],
                    in_=wsrc.rearrange("(c p) d -> p c d", p=P)
                            [:, :, m * 2 * P:(m + span) * 2 * P])

            nc.vector.memset(junk[:], 0.5)   # first: unblocks PE warm-up
            natq = load_nat(dq, "q", (0,))
            # identity built right after the first input DMA is queued:
            # its Pool-engine ops must not queue behind all the SWDGE
            # generations, or the first transposes gate on the identity
            make_identity(nc, ident_bf)
            natq += load_nat(dq, "q", (1, 2, 3))
            nc.vector.memset(ones_bf[:], 1.0)
            nc.vector.memset(ebias[:], EXP_BIAS)
            # tiny Copy activation: forces the copy+exp act table to load
            # during startup idle, so the tail's ScalarE norm needs no swap
            nc.scalar.copy(actwarm[:], ebias[0:1, 0:1])
            load_w_slice(wqa, dwq, 0)
            natk = load_nat(dk_, "k", (0, 1, 2, 3))
            load_w_slice(wka, dwk, 0)
            natv = load_nat(dv, "v", (0, 1, 2, 3))
            load_w_slice(wva, dwv, 0)
            load_w_slice(wqa, dwq, 1)
            load_w_slice(wka, dwk, 1)
            load_w_slice(wva, dwv, 1)
            for m in range(2, 4):
                load_w_slice(wqa, dwq, m)
                load_w_slice(wka, dwk, m)
                load_w_slice(wva, dwv, m)
            if use_bias:
                nc.sync.dma_start(bqt[:], dbq.rearrange("(c p) -> p c", p=P))
                nc.sync.dma_start(bkt[:], dbk.rearrange("(c p) -> p c", p=P))
                nc.gpsimd.dma_start(out=bvr[:], in_=dbv[None, :])

            # ---- PE warm-up: junk matmuls walk the p-state up while the
            # first input DMAs land ----
            with tc.tile_pool(name="warm_ps", bufs=1, space="PSUM") as wps:
                for i in range(28):
                    wt = wps.tile([P, DK], F32, name=f"wpsum{i}", tag="warm",
                                  bufs=2)
                    nc.tensor.matmul(wt[0:DK, :], junk[:], junk[:],
                                     start=True, stop=True)

            sc = None  # created after the transpose pool is released

            def tr_chunk(dst, natt, rr, nm, pool, tag="tp"):
                for r2 in range(2):
                    tpt = pool.tile([P, NB, P], BF16,
                                    name=f"tpt_{nm}_{rr}_{r2}",
                                    tag=tag, bufs=2)
                    for c in range(NB):
                        nc.tensor.transpose(
                            tpt[:, c, :], natt[:, r2, c * P:(c + 1) * P],
                            ident_bf[:])
                    nc.vector.tensor_copy(
                        dst[:, :, (rr + r2) * P:(rr + r2 + 1) * P], tpt[:])

            cur = {}   # live per-pair tiles: (kind, p) -> AP

            def qk_unit_q(p, tens, q):
                """QT/KT[p][:, q*256:(q+1)*256] = ((x @ w)^T + b) quarter:
                0.85us quantum for slot-level smoothing."""
                key = ("QT" if tens == 0 else "KT", p)
                if key not in cur:
                    cur[key] = ring.tile([P, S], F32R, name=f"{key[0]}{p}",
                                         tag=key[0], bufs=2)
                dst = cur[key]
                wsrc = wqa if tens == 0 else wka
                bsrc = bqt if tens == 0 else bkt
                xsrc = qt if tens == 0 else kt
                QTR = S // 4
                pj = scr.tile([P, QTR], F32, name=f"pj_{p}_{tens}_{q}",
                              tag="scr", bufs=2)
                for kc in range(NB):
                    nc.tensor.matmul(pj[:], wsrc[:, kc, p * P:(p + 1) * P],
                                     xsrc[:, kc, q * QTR:(q + 1) * QTR],
                                     start=(kc == 0), stop=(kc == NB - 1))
                if use_bias:
                    nc.vector.tensor_scalar_add(
                        dst[:, q * QTR:(q + 1) * QTR], pj[:],
                        bsrc[:, p:p + 1])
                else:
                    nc.vector.tensor_copy(
                        dst[:, q * QTR:(q + 1) * QTR], pj[:])

            def qk_unit(p, tens, hf):
                qk_unit_q(p, tens, hf * 2)
                qk_unit_q(p, tens, hf * 2 + 1)

            def v_unit_q(p, qr):
                """VA[p][:, 2 j-blocks, head, 0:64] for quarter qr (2 of 8
                s-blocks): 0.85us quantum for slot-level smoothing."""
                key = ("VA", p)
                if key not in cur:
                    cur[key] = ring.tile([P, NB, 2, DK + 1], PVDT,
                                         name=f"VA{p}", tag="VA", bufs=2)
                    nc.vector.memset(cur[key][:, :, :, DK:DK + 1], 1.0)
                va = cur[key]
                pjv = scr.tile([P, 2 * P], F32, name=f"pjv_{p}_{qr}",
                               tag="scr", bufs=2)
                pv4 = pjv.rearrange("q (r f) -> q r f", f=P)
                for r4 in range(2):
                    r = qr * 2 + r4
                    for kc in range(NB):
                        nc.tensor.matmul(
                            pv4[:, r4, :], vt[:, kc, r * P:(r + 1) * P],
                            wva[:, kc, p * P:(p + 1) * P],
                            start=(kc == 0),
                            stop=(not use_bias and kc == NB - 1))
                    if use_bias:
                        nc.tensor.matmul(pv4[:, r4, :], ones_bf[:],
                                         bvr[:, p * P:(p + 1) * P],
                                         start=False, stop=True)
                nc.vector.tensor_copy(
                    va[:, qr * 2:(qr + 1) * 2, :, 0:DK],
                    pjv.rearrange("q (r hh f) -> q r hh f", r=2, hh=2))

            def v_unit(p, half):
                v_unit_q(p, half * 2)
                v_unit_q(p, half * 2 + 1)

            def score_slot(p, hh, jb):
                """scores^T for one (head, j-block) -> exp -> eT."""
                QTp, KTp = cur[("QT", p)], cur[("KT", p)]
                eTp = cur[("eT", p)]
                sct = sc.tile([P, S], F32, name=f"sc_{p}_{hh}_{jb}",
                              tag="sc", bufs=3)
                for ih in range(2):
                    nc.tensor.matmul(
                        sct[:, ih * HALF:(ih + 1) * HALF],
                        KTp[hh * DK:(hh + 1) * DK, jb * P:(jb + 1) * P],
                        QTp[hh * DK:(hh + 1) * DK,
                            ih * HALF:(ih + 1) * HALF],
                        start=True, stop=True)
                nc.scalar.activation(
                    out=eTp[:, jb, hh, :], in_=sct[:],
                    func=EXP, scale=EXP_SCALE, bias=ebias[:])

            def pv_mms(pu4, eTp, VAp, hh, half, jbs, start, stop):
                # ib-major: a PSUM bank must hold only ONE open accumulation
                # group at a time (start_tensor_calc zeroing is bank-granular)
                for i4 in range(4):
                    ib = half * 4 + i4
                    for jb in jbs:
                        nc.tensor.matmul(
                            pu4[:, i4, 0:DK + 1],
                            eTp[:, jb, hh, ib * P:(ib + 1) * P],
                            VAp[:, jb, hh, :],
                            start=(jb == jbs[0] and start),
                            stop=(jb == jbs[-1] and stop))

            def pv_norm(pu4, stage, hh, half):
                rcpt = ring.tile([P, 4], F32, name=f"rcp_{hh}_{half}",
                                 tag="rcp", bufs=4)
                nc.vector.reciprocal(rcpt[:], pu4[:, :, DK])
                for i4 in range(4):
                    nc.vector.tensor_scalar_mul(
                        stage[:, half * 4 + i4, hh * DK:(hh + 1) * DK],
                        pu4[:, i4, 0:DK], rcpt[:, i4:i4 + 1])

            def pv_unit(p, hh, half, stage):
                """out[i, 0:65] for 4 i-blocks; normalize into stage."""
                eTp, VAp = cur[("eT", p)], cur[("VA", p)]
                pu = scr.tile([P, HALF], F32, name=f"pu_{p}_{hh}_{half}",
                              tag="scr", bufs=2)
                pu4 = pu.rearrange("q (i f) -> q i f", f=P)
                if pv_fp8 and not _NO_DR:
                    for i4 in range(4):
                        for t in range(4):
                            nc.tensor.matmul(
                                pu4[:, i4, 0:DK + 1],
                                eTp[:, 2 * t:2 * t + 2, hh,
                                    (half * 4 + i4) * P:(half * 4 + i4 + 1) * P],
                                VAp[:, 2 * t:2 * t + 2, hh, :],
                                start=(t == 0), stop=(t == 3), perf_mode=DR)
                else:
                    pv_mms(pu4, eTp, VAp, hh, half, list(range(NB)),
                           True, True)
                pv_norm(pu4, stage, hh, half)

            def out_dma(p, stage):
                nc.sync.dma_start(
                    dout[:, p * P:(p + 1) * P]
                    .rearrange("(ib q) c -> q ib c", q=P), stage[:])

            def out_dma_part(p, stage, hh, half, engine):
                engine.dma_start(
                    dout[half * HALF:(half + 1) * HALF,
                         p * P + hh * DK:p * P + (hh + 1) * DK]
                    .rearrange("(ib q) c -> q ib c", q=P),
                    stage[:, half * 4:(half + 1) * 4, hh * DK:(hh + 1) * DK])

            # ---- startup: all transposes (scoped PSUM pool), then pair-0
            # QK projections; the engines' wait-queue bypass interleaves
            # the projection matmuls under the DMA-paced transposes ----
            with tc.tile_pool(name="tp_ps", bufs=1, space="PSUM") as tps:
                for rr in range(0, NB, 2):
                    tr_chunk(qt, natq[rr // 2], rr, "q", tps)
                qk_unit(0, 0, 0)
                qk_unit(0, 0, 1)
                # pair-1 projection quarters fill the DMA-paced stalls
                # between transpose chunks (wq0/wk0 slices cover pair 1 too)
                for rr in range(0, NB, 2):
                    tr_chunk(kt, natk[rr // 2], rr, "k", tps)
                    qk_unit_q(1, 0, rr // 2)
                qk_unit(0, 1, 0)
                qk_unit(0, 1, 1)
                # v transposes move into pair-0's Act-bound slots: nothing
                # else may delay the first score slot

            sc = ctx.enter_context(tc.tile_pool(name="sc", bufs=1,
                                                space="PSUM"))

            stages = {}

            def ensure_pair(p):
                if ("eT", p) not in cur:
                    cur[("eT", p)] = ring.tile([P, NB, 2, S], PVDT,
                                               name=f"eT{p}", tag="eT",
                                               bufs=2)
                if p not in stages:
                    stages[p] = ring.tile([P, NB, P], F32,
                                          name=f"stage{p}", tag="stage",
                                          bufs=2)

            def run(steps):
                for st in steps:
                    if st[0] == "sc":
                        ensure_pair(st[1])
                        score_slot(st[1], st[2], st[3])
                    else:
                        st[1]()

            # ---- pair 0 hand-scheduled: the v transposes (DMA-paced) and
            # pair-1's K projections ride its Act-bound slots ----
            ensure_pair(0)
            run([
                ("sc", 0, 0, 0),
                ("f", lambda: tr_chunk(vt, natv[0], 0, "v", scr, tag="scr")),
                ("sc", 0, 0, 1), ("f", lambda: qk_unit_q(1, 1, 0)),
                ("sc", 0, 0, 2),
                ("f", lambda: tr_chunk(vt, natv[1], 2, "v", scr, tag="scr")),
                ("sc", 0, 0, 3), ("f", lambda: qk_unit_q(1, 1, 1)),
                ("sc", 0, 0, 4),
                ("f", lambda: tr_chunk(vt, natv[2], 4, "v", scr, tag="scr")),
                ("sc", 0, 0, 5), ("f", lambda: qk_unit_q(1, 1, 2)),
                ("sc", 0, 0, 6),
                ("f", lambda: tr_chunk(vt, natv[3], 6, "v", scr, tag="scr")),
                ("sc", 0, 0, 7), ("f", lambda: qk_unit_q(1, 1, 3)),
                ("sc", 0, 1, 0), ("f", lambda: v_unit_q(0, 0)),
                ("sc", 0, 1, 1), ("f", lambda: v_unit_q(0, 1)),
                ("sc", 0, 1, 2), ("f", lambda: v_unit_q(0, 2)),
                ("sc", 0, 1, 3), ("f", lambda: v_unit_q(0, 3)),
                ("sc", 0, 1, 4), ("f", lambda: pv_unit(0, 0, 0, stages[0])),
                ("sc", 0, 1, 5), ("f", lambda: pv_unit(0, 0, 1, stages[0])),
                ("sc", 0, 1, 6), ("f", lambda: v_unit_q(1, 0)),
                ("sc", 0, 1, 7), ("f", lambda: v_unit_q(1, 1)),
            ])
            for k in (("QT", 0), ("KT", 0)):
                cur.pop(k, None)

            # ---- pairs 1..4: the original per-pair software pipeline ----
            for p in range(1, NPAIR - 3):
                ensure_pair(p)
                nxt = p + 1
                fillers = {}
                # slot-0/1 fills must not depend on the previous stretch's
                # last exp (junction stall): pv(p-1,1) sits a few slots in
                def _pv1a():
                    pv_unit(p - 1, 1, 0, stages[p - 1])
                def _pv1b():
                    pv_unit(p - 1, 1, 1, stages[p - 1])
                    out_dma(p - 1, stages[p - 1])
                if p == 1:
                    fillers[0] = lambda: v_unit_q(1, 2)
                    fillers[1] = lambda: v_unit_q(1, 3)
                    fillers[3] = _pv1a
                    fillers[5] = _pv1b
                    qslots = (2, 4, 6, 8)
                    fillers[qslots[0]] = lambda: qk_unit(nxt, 0, 0)
                    fillers[qslots[1]] = lambda: qk_unit(nxt, 0, 1)
                    fillers[qslots[2]] = lambda: qk_unit(nxt, 1, 0)
                    fillers[qslots[3]] = lambda: qk_unit(nxt, 1, 1)
                    fillers[10] = lambda: v_unit(nxt, 0)
                    fillers[14] = lambda: v_unit(nxt, 1)
                else:
                    # quarter-granular fills, one per slot; pv(p-1,1) at 6/7
                    # so the VA-ring WAR clears before the v quarters at 10+
                    for si_, qi in ((0, 0), (1, 1), (2, 2), (3, 3), (4, 4),
                                    (5, 5), (8, 6), (9, 7)):
                        fillers[si_] = (lambda qi=qi:
                                        qk_unit_q(nxt, qi // 4, qi % 4))
                    fillers[6] = _pv1a
                    fillers[7] = _pv1b
                    fillers[10] = lambda: v_unit_q(nxt, 0)
                    fillers[11] = lambda: v_unit_q(nxt, 1)
                    fillers[14] = lambda: v_unit_q(nxt, 2)
                    fillers[15] = lambda: v_unit_q(nxt, 3)
                fillers[12] = lambda: pv_unit(p, 0, 0, stages[p])
                fillers[13] = lambda: pv_unit(p, 0, 1, stages[p])
                si = 0
                for hh in range(2):
                    for jb in range(NB):
                        score_slot(p, hh, jb)
                        if si in fillers:
                            fillers[si]()
                        si += 1
                for k in (("QT", p), ("KT", p)):
                    cur.pop(k, None)

            # ---- pairs 5..7: hand-scheduled with run-ahead score slots.
            # Act (exp) needs 16.7us per 16 slots but the last pairs have
            # too little PE filler work, so pair-6 h0 scores start inside
            # pair-5's stretch and pair-7 h0 scores inside pair-6's: the
            # Act-bound final phase shrinks by the borrowed slots. ----
            ensure_pair(5)
            run([
                ("sc", 5, 0, 0), ("f", lambda: qk_unit_q(6, 0, 0)),
                ("sc", 5, 0, 1), ("f", lambda: qk_unit_q(6, 0, 1)),
                ("sc", 5, 0, 2), ("f", lambda: qk_unit_q(6, 0, 2)),
                ("sc", 5, 0, 3), ("f", lambda: qk_unit_q(6, 0, 3)),
                ("sc", 5, 0, 4), ("f", lambda: qk_unit_q(6, 1, 0)),
                ("sc", 5, 0, 5), ("f", lambda: qk_unit_q(6, 1, 1)),
                ("sc", 5, 0, 6), ("f", lambda: qk_unit_q(6, 1, 2)),
                ("sc", 5, 0, 7), ("f", lambda: qk_unit_q(6, 1, 3)),
                ("sc", 5, 1, 0), ("f", lambda: pv_unit(4, 1, 0, stages[4])),
                ("sc", 5, 1, 1),
                ("f", lambda: (pv_unit(4, 1, 1, stages[4]),
                               out_dma(4, stages[4]))),
                ("sc", 5, 1, 2), ("f", lambda: v_unit_q(6, 0)),
                ("sc", 6, 0, 0),
                ("sc", 5, 1, 3), ("f", lambda: v_unit_q(6, 1)),
                ("sc", 6, 0, 1),
                ("sc", 5, 1, 4), ("f", lambda: pv_unit(5, 0, 0, stages[5])),
                ("sc", 6, 0, 2),
                ("sc", 5, 1, 5), ("f", lambda: pv_unit(5, 0, 1, stages[5])),
                ("sc", 6, 0, 3),
                ("sc", 5, 1, 6), ("f", lambda: v_unit_q(6, 2)),
                ("sc", 5, 1, 7), ("f", lambda: v_unit_q(6, 3)),
            ])
            for k in (("QT", 5), ("KT", 5)):
                cur.pop(k, None)

            # pair-6 stretch absorbs ALL of pair-7's h0 scores and the first
            # two h1 scores: Act saturates from here to the end while PE
            # (balanced ~23us work vs ~22.8us Act) keeps feeding it
            run([
                ("sc", 6, 0, 4), ("f", lambda: qk_unit_q(7, 0, 0)),
                ("sc", 6, 0, 5), ("f", lambda: qk_unit_q(7, 0, 1)),
                ("sc", 6, 0, 6), ("f", lambda: qk_unit_q(7, 0, 2)),
                ("sc", 6, 0, 7), ("f", lambda: qk_unit_q(7, 0, 3)),
                ("sc", 6, 1, 0), ("f", lambda: qk_unit_q(7, 1, 0)),
                ("sc", 6, 1, 1), ("f", lambda: qk_unit_q(7, 1, 1)),
                ("sc", 6, 1, 2), ("f", lambda: qk_unit_q(7, 1, 2)),
                ("sc", 6, 1, 3), ("f", lambda: qk_unit_q(7, 1, 3)),
                ("sc", 6, 1, 4), ("f", lambda: pv_unit(5, 1, 0, stages[5])),
                ("sc", 6, 1, 5),
                ("f", lambda: (pv_unit(5, 1, 1, stages[5]),
                               out_dma(5, stages[5]))),
                ("sc", 6, 1, 6), ("f", lambda: pv_unit(6, 0, 0, stages[6])),
                ("sc", 7, 0, 0),
                ("sc", 6, 1, 7), ("f", lambda: pv_unit(6, 0, 1, stages[6])),
                ("sc", 7, 0, 1),
                ("sc", 7, 0, 2), ("f", lambda: v_unit_q(7, 0)),
                ("sc", 7, 0, 3),
                ("sc", 7, 0, 4), ("f", lambda: v_unit_q(7, 1)),
                ("sc", 7, 0, 5),
                ("sc", 7, 0, 6), ("f", lambda: pv_unit(6, 1, 0, stages[6])),
                ("sc", 7, 0, 7),
                ("sc", 7, 1, 0),
                ("f", lambda: (pv_unit(6, 1, 1, stages[6]),
                               out_dma(6, stages[6]))),
                ("sc", 7, 1, 1),
            ])
            for k in (("QT", 6), ("KT", 6)):
                cur.pop(k, None)

            # final stretch: the last six scores issue with nothing heavy in
            # front of them; all deferrable PV/V work runs AFTER the last
            # score, overlapping the final exps
            run([
                ("sc", 7, 1, 2),
                ("sc", 7, 1, 3), ("f", lambda: v_unit_q(7, 2)),
                ("sc", 7, 1, 4), ("f", lambda: v_unit_q(7, 3)),
                ("sc", 7, 1, 5), ("f", lambda: pv_unit(7, 0, 0, stages[7])),
                ("sc", 7, 1, 6),
                ("f", lambda: (pv_unit(7, 0, 1, stages[7]),
                               out_dma_part(7, stages[7], 0, 0, nc.sync),
                               out_dma_part(7, stages[7], 0, 1, nc.sync))),
                ("sc", 7, 1, 7),
            ])
            for k in (("QT", 7), ("KT", 7)):
                cur.pop(k, None)

            # tail: final head's PV; h0 normalizes on DVE, h1 on the (now
            # idle) ScalarE so the two norm chains overlap; DMAs on
            # different queues
            stage = stages[NPAIR - 1]
            eTp7, VAp7 = cur[("eT", NPAIR - 1)], cur[("VA", NPAIR - 1)]
            pus = []
            for half in range(2):
                pu = scr.tile([P, HALF], F32, name=f"pu_t_{half}",
                              tag="scr", bufs=2)
                pu4 = pu.rearrange("q (i f) -> q i f", f=P)
                pv_mms(pu4, eTp7, VAp7, 1, half, list(range(NB)),
                       True, True)
                pus.append(pu4)
            for half in range(2):
                rcpt = ring.tile([P, 4], F32, name=f"rcp_t_{half}",
                                 tag="rcp", bufs=4)
                nc.vector.reciprocal(rcpt[:], pus[half][:, :, DK])
                for i4 in range(4):
                    if half == 0:
                        nc.vector.tensor_scalar_mul(
                            stage[:, i4, DK:2 * DK],
                            pus[0][:, i4, 0:DK], rcpt[:, i4:i4 + 1])
                    else:
                        nc.scalar.mul(
                            stage[:, 4 + i4, DK:2 * DK],
                            pus[1][:, i4, 0:DK], rcpt[:, i4:i4 + 1])
                out_dma_part(NPAIR - 1, stage, 1, half,
                             nc.sync if half == 0 else nc.scalar)

    nc.compile()
    return nc


def kernel(q, k, v, w_q, b_q, w_k, b_k, w_v, b_v):
    use_bias = bool(np.any(np.asarray(b_q)) or np.any(np.asarray(b_k))
                    or np.any(np.asarray(b_v)))
    key = (use_bias, PV_FP8, _NO_DR)
    if key not in _compiled:
        _compiled[key] = _build(use_bias, PV_FP8)
    nc = _compiled[key]

    f = lambda x: np.ascontiguousarray(np.asarray(x, dtype=np.float32))
    in_maps = []
    for c in range(N_CORES):
        in_maps.append({
            "q": f(q[c]), "k": f(k[c]), "v": f(v[c]),
            "wq": f(w_q), "wk": f(w_k), "wv": f(w_v),
            "bq": f(b_q), "bk": f(b_k), "bv": f(b_v),
        })
    res = run_bass_kernel_spmd(nc, in_maps, list(range(N_CORES)))
    out = np.stack([res.results[c]["out"] for c in range(N_CORES)], axis=0)
    kernel.last_results = res
    return out



# revision 69
# speedup vs baseline: 1.0055x; 1.0055x over previous
"""Multi-head attention (B=8, S=1024, D=1024, H=16) on 8 TRN2 NeuronCores.

Sharding: pure data-parallel over batch - each core computes one batch
element end-to-end (weights replicated per core), so no collectives.

Per-core pipeline (PE-bound design: ~177us of matmul rows at bf16, with
softmax exp on ScalarE (~133us) hidden underneath):
  1. Inputs DMA'd with f32->bf16 cast (SWDGE), ordered on the serial DMA
     FIFO so pair-0's projections start ~15us in; weights load into
     persistent tiles in per-2-pair column slices. A burst of junk
     matmuls at t~1us walks the PE p-state up to full clock before real
     work arrives.
  2. PE-transposes pack inputs into [P, kc, S] (contraction on
     partitions) via a scoped 2-bank PSUM pool; pair-0's QK projection
     matmuls slip underneath the DMA-paced transposes through the
     engines' 4-deep wait-queue bypass.
  3. Per head-pair p: QT/KT[p] = (x @ w)^T in f32 (f32r matmuls, bias
     folded into the PSUM drain); V natural orientation into VA[p]
     [j_part, jb, head, 65] (bf16) with a ones column.
  4. 16 score/exp slots per pair (head x j-block): scores^T[j,i] via 2
     f32r matmuls into a [P, S] PSUM tile (2 banks, bufs=3 ring); one
     exp(s/8 - 2.4) per slot on ScalarE straight out of PSUM -> bf16
     eT[p] [j_part, jb, head, i].
  5. PV natural: out[i, dk+1] accumulated over jb (lhsT = eT block,
     rhs = VA block); col 64 = sum(exp) via the ones column. Reciprocal
     + per-partition-scalar normalize into a per-pair staging tile, one
     [P, ib, 128] DMA per pair (split per head at the tail).
  6. Emission is software-pipelined: projections for pair p+1, PV of
     pair p (h0) and of pair p-1 (h1) ride in pair p's slots so the
     in-order PE queue never head-of-line blocks.
     NOTE: a PSUM bank must only ever hold ONE open matmul accumulation
     group (start_tensor_calc zeroing is region-granular), so PV is
     strictly ib-major.
  7. End-game scheduling (the kernel's finish = Act-stream end + tail,
     and Act saturates ~25us before the end): fills are split into
     ~0.85us quanta (qk/v quarter units) and spread one per score slot;
     pair-6's stretch absorbs ALL of pair-7's h0 score slots plus
     (7,1,0..1) as run-ahead (eT ring bufs=2 allows exactly one pair of
     look-ahead); the final six scores issue with nothing in front of
     them and all deferrable PV/V work runs after them, overlapping the
     last exps via the framework's fine-grained per-matmul waits.
     Pair-0's Act-bound slots host the v transposes + pair-1's late
     projection quarters, pulling the first score slot ~7us earlier.
     Stretch-junction fills must not depend on the previous stretch's
     last exp (pv(p-1,h1) sits at slots 8/9, not 0/1).
     Baseline 199564ns -> 195071ns (TimelineSim; identical numerics).
"""
import numpy as np
from contextlib import ExitStack

import concourse.bass as bass
import concourse.mybir as mybir
import concourse.tile as tile
from concourse import bacc
from concourse.bass_utils import run_bass_kernel_spmd
from concourse.masks import make_identity

F32 = mybir.dt.float32
F32R = mybir.dt.float32r
BF16 = mybir.dt.bfloat16
FP8 = mybir.dt.float8e4
DR = mybir.MatmulPerfMode.DoubleRow
EXP = mybir.ActivationFunctionType.Exp

B, S, D, H, DK = 8, 1024, 1024, 16, 64
P = 128
NB = S // P            # 8 row/col blocks
NPAIR = H // 2         # 8 head pairs
HALF = 512
N_CORES = 8
EXP_SCALE = 0.125      # 1/sqrt(dk)
EXP_BIAS = -2.4        # keeps bf16 exp well-scaled; fp8 would need <240

PV_FP8 = False         # probs/V in fp8e4 + DoubleRow PV (else bf16).
                       # fp8 measured 6e-2 rel err: attention rows here are
                       # concentrated, so 3.6% fp8 noise on dominant probs
                       # does not average out. bf16 keeps 7e-3.
_NO_DR = False         # debug: fp8 without DoubleRow perf mode
TAIL_JB_MAJOR = False  # pipeline the final head's PV across exp slots

_compiled = {}


def _build(use_bias=True, pv_fp8=PV_FP8):
    nc = bacc.Bacc("TRN2", target_bir_lowering=False, debug=False,
                   enable_asserts=False, num_devices=N_CORES)

    dq = nc.dram_tensor("q", [S, D], F32, kind="ExternalInput").ap()
    dk_ = nc.dram_tensor("k", [S, D], F32, kind="ExternalInput").ap()
    dv = nc.dram_tensor("v", [S, D], F32, kind="ExternalInput").ap()
    dwq = nc.dram_tensor("wq", [D, D], F32, kind="ExternalInput").ap()
    dwk = nc.dram_tensor("wk", [D, D], F32, kind="ExternalInput").ap()
    dwv = nc.dram_tensor("wv", [D, D], F32, kind="ExternalInput").ap()
    dbq = nc.dram_tensor("bq", [D], F32, kind="ExternalInput").ap()
    dbk = nc.dram_tensor("bk", [D], F32, kind="ExternalInput").ap()
    dbv = nc.dram_tensor("bv", [D], F32, kind="ExternalInput").ap()
    dout = nc.dram_tensor("out", [S, D], F32, kind="ExternalOutput").ap()

    PVDT = FP8 if pv_fp8 else BF16

    with tile.TileContext(nc) as tc:
        with ExitStack() as ctx:
            const = ctx.enter_context(tc.tile_pool(name="const", bufs=1))
            persist = ctx.enter_context(tc.tile_pool(name="persist", bufs=1))
            ring = ctx.enter_context(tc.tile_pool(name="ring", bufs=1))
            scr = ctx.enter_context(tc.tile_pool(name="scr", bufs=1,
                                                 space="PSUM"))

            ident_bf = const.tile([P, P], BF16)
            ones_bf = const.tile([1, P], BF16)
            ebias = const.tile([P, 1], F32)
            junk = const.tile([P, DK], BF16)

            qt = persist.tile([P, NB, S], BF16, name="qt")
            kt = persist.tile([P, NB, S], BF16, name="kt")
            vt = persist.tile([P, NB, S], BF16, name="vt")
            wqa = persist.tile([P, NB, D], BF16, name="wqa")
            wka = persist.tile([P, NB, D], BF16, name="wka")
            wva = persist.tile([P, NB, D], BF16, name="wva")
            bqt = persist.tile([P, NPAIR], F32, name="bqt")
            bkt = persist.tile([P, NPAIR], F32, name="bkt")
            bvr = persist.tile([1, D], BF16, name="bvr")

            # ---- DMA FIFO order tuned so pair-0's QK projections can
            # start as early as possible on the serial DMA device ----
            def load_nat(dsrc, nm, chunks):
                tiles = []
                for ci in chunks:
                    rr = ci * 2
                    natt = ring.tile([P, 2, S], BF16, name=f"nat_{nm}_{rr}",
                                     tag="nat", bufs=4)
                    nc.gpsimd.dma_start(
                        out=natt[:],
                        in_=dsrc[rr * P:(rr + 2) * P, :]
                            .rearrange("(r p) d -> p r d", p=P))
                    tiles.append(natt)
                return tiles

            def load_w_slice(wdst, wsrc, m, span=1):
                nc.gpsimd.dma_start(
                    out=wdst[:, :, m * 2 * P:(m + span) * 2 * P],
                    in_=wsrc.rearrange("(c p) d -> p c d", p=P)
                            [:, :, m * 2 * P:(m + span) * 2 * P])

            nc.vector.memset(junk[:], 0.5)   # first: unblocks PE warm-up
            natq = load_nat(dq, "q", (0,))
            # identity built right after the first input DMA is queued:
            # its Pool-engine ops must not queue behind all the SWDGE
            # generations, or the first transposes gate on the identity
            make_identity(nc, ident_bf)
            natq += load_nat(dq, "q", (1, 2, 3))
            nc.vector.memset(ones_bf[:], 1.0)
            nc.vector.memset(ebias[:], EXP_BIAS)
            load_w_slice(wqa, dwq, 0)
            natk = load_nat(dk_, "k", (0, 1, 2, 3))
            load_w_slice(wka, dwk, 0)
            natv = load_nat(dv, "v", (0, 1, 2, 3))
            load_w_slice(wva, dwv, 0)
            load_w_slice(wqa, dwq, 1)
            load_w_slice(wka, dwk, 1)
            load_w_slice(wva, dwv, 1)
            for m in range(2, 4):
                load_w_slice(wqa, dwq, m)
                load_w_slice(wka, dwk, m)
                load_w_slice(wva, dwv, m)
            if use_bias:
                nc.sync.dma_start(bqt[:], dbq.rearrange("(c p) -> p c", p=P))
                nc.sync.dma_start(bkt[:], dbk.rearrange("(c p) -> p c", p=P))
                nc.gpsimd.dma_start(out=bvr[:], in_=dbv[None, :])

            # ---- PE warm-up: junk matmuls walk the p-state up while the
            # first input DMAs land ----
            with tc.tile_pool(name="warm_ps", bufs=1, space="PSUM") as wps:
                for i in range(28):
                    wt = wps.tile([P, DK], F32, name=f"wpsum{i}", tag="warm",
                                  bufs=2)
                    nc.tensor.matmul(wt[0:DK, :], junk[:], junk[:],
                                     start=True, stop=True)

            sc = None  # created after the transpose pool is released

            def tr_chunk(dst, natt, rr, nm, pool, tag="tp"):
                for r2 in range(2):
                    tpt = pool.tile([P, NB, P], BF16,
                                    name=f"tpt_{nm}_{rr}_{r2}",
                                    tag=tag, bufs=2)
                    for c in range(NB):
                        nc.tensor.transpose(
                            tpt[:, c, :], natt[:, r2, c * P:(c + 1) * P],
                            ident_bf[:])
                    nc.vector.tensor_copy(
                        dst[:, :, (rr + r2) * P:(rr + r2 + 1) * P], tpt[:])

            cur = {}   # live per-pair tiles: (kind, p) -> AP

            def qk_unit_q(p, tens, q):
                """QT/KT[p][:, q*256:(q+1)*256] = ((x @ w)^T + b) quarter:
                0.85us quantum for slot-level smoothing."""
                key = ("QT" if tens == 0 else "KT", p)
                if key not in cur:
                    cur[key] = ring.tile([P, S], F32R, name=f"{key[0]}{p}",
                                         tag=key[0], bufs=2)
                dst = cur[key]
                wsrc = wqa if tens == 0 else wka
                bsrc = bqt if tens == 0 else bkt
                xsrc = qt if tens == 0 else kt
                QTR = S // 4
                pj = scr.tile([P, QTR], F32, name=f"pj_{p}_{tens}_{q}",
                              tag="scr", bufs=2)
                for kc in range(NB):
                    nc.tensor.matmul(pj[:], wsrc[:, kc, p * P:(p + 1) * P],
                                     xsrc[:, kc, q * QTR:(q + 1) * QTR],
                                     start=(kc == 0), stop=(kc == NB - 1))
                if use_bias:
                    nc.vector.tensor_scalar_add(
                        dst[:, q * QTR:(q + 1) * QTR], pj[:],
                        bsrc[:, p:p + 1])
                else:
                    nc.vector.tensor_copy(
                        dst[:, q * QTR:(q + 1) * QTR], pj[:])

            def qk_unit(p, tens, hf):
                qk_unit_q(p, tens, hf * 2)
                qk_unit_q(p, tens, hf * 2 + 1)

            def v_unit_q(p, qr):
                """VA[p][:, 2 j-blocks, head, 0:64] for quarter qr (2 of 8
                s-blocks): 0.85us quantum for slot-level smoothing."""
                key = ("VA", p)
                if key not in cur:
                    cur[key] = ring.tile([P, NB, 2, DK + 1], PVDT,
                                         name=f"VA{p}", tag="VA", bufs=2)
                    nc.vector.memset(cur[key][:, :, :, DK:DK + 1], 1.0)
                va = cur[key]
                pjv = scr.tile([P, 2 * P], F32, name=f"pjv_{p}_{qr}",
                               tag="scr", bufs=2)
                pv4 = pjv.rearrange("q (r f) -> q r f", f=P)
                for r4 in range(2):
                    r = qr * 2 + r4
                    for kc in range(NB):
                        nc.tensor.matmul(
                            pv4[:, r4, :], vt[:, kc, r * P:(r + 1) * P],
                            wva[:, kc, p * P:(p + 1) * P],
                            start=(kc == 0),
                            stop=(not use_bias and kc == NB - 1))
                    if use_bias:
                        nc.tensor.matmul(pv4[:, r4, :], ones_bf[:],
                                         bvr[:, p * P:(p + 1) * P],
                                         start=False, stop=True)
                nc.vector.tensor_copy(
                    va[:, qr * 2:(qr + 1) * 2, :, 0:DK],
                    pjv.rearrange("q (r hh f) -> q r hh f", r=2, hh=2))

            def v_unit(p, half):
                v_unit_q(p, half * 2)
                v_unit_q(p, half * 2 + 1)

            def score_slot(p, hh, jb):
                """scores^T for one (head, j-block) -> exp -> eT."""
                QTp, KTp = cur[("QT", p)], cur[("KT", p)]
                eTp = cur[("eT", p)]
                sct = sc.tile([P, S], F32, name=f"sc_{p}_{hh}_{jb}",
                              tag="sc", bufs=3)
                for ih in range(2):
                    nc.tensor.matmul(
                        sct[:, ih * HALF:(ih + 1) * HALF],
                        KTp[hh * DK:(hh + 1) * DK, jb * P:(jb + 1) * P],
                        QTp[hh * DK:(hh + 1) * DK,
                            ih * HALF:(ih + 1) * HALF],
                        start=True, stop=True)
                nc.scalar.activation(
                    out=eTp[:, jb, hh, :], in_=sct[:],
                    func=EXP, scale=EXP_SCALE, bias=ebias[:])

            def pv_mms(pu4, eTp, VAp, hh, half, jbs, start, stop):
                # ib-major: a PSUM bank must hold only ONE open accumulation
                # group at a time (start_tensor_calc zeroing is bank-granular)
                for i4 in range(4):
                    ib = half * 4 + i4
                    for jb in jbs:
                        nc.tensor.matmul(
                            pu4[:, i4, 0:DK + 1],
                            eTp[:, jb, hh, ib * P:(ib + 1) * P],
                            VAp[:, jb, hh, :],
                            start=(jb == jbs[0] and start),
                            stop=(jb == jbs[-1] and stop))

            def pv_norm(pu4, stage, hh, half):
                rcpt = ring.tile([P, 4], F32, name=f"rcp_{hh}_{half}",
                                 tag="rcp", bufs=4)
                nc.vector.reciprocal(rcpt[:], pu4[:, :, DK])
                for i4 in range(4):
                    nc.vector.tensor_scalar_mul(
                        stage[:, half * 4 + i4, hh * DK:(hh + 1) * DK],
                        pu4[:, i4, 0:DK], rcpt[:, i4:i4 + 1])

            def pv_unit(p, hh, half, stage):
                """out[i, 0:65] for 4 i-blocks; normalize into stage."""
                eTp, VAp = cur[("eT", p)], cur[("VA", p)]
                pu = scr.tile([P, HALF], F32, name=f"pu_{p}_{hh}_{half}",
                              tag="scr", bufs=2)
                pu4 = pu.rearrange("q (i f) -> q i f", f=P)
                if pv_fp8 and not _NO_DR:
                    for i4 in range(4):
                        for t in range(4):
                            nc.tensor.matmul(
                                pu4[:, i4, 0:DK + 1],
                                eTp[:, 2 * t:2 * t + 2, hh,
                                    (half * 4 + i4) * P:(half * 4 + i4 + 1) * P],
                                VAp[:, 2 * t:2 * t + 2, hh, :],
                                start=(t == 0), stop=(t == 3), perf_mode=DR)
                else:
                    pv_mms(pu4, eTp, VAp, hh, half, list(range(NB)),
                           True, True)
                pv_norm(pu4, stage, hh, half)

            def out_dma(p, stage):
                nc.sync.dma_start(
                    dout[:, p * P:(p + 1) * P]
                    .rearrange("(ib q) c -> q ib c", q=P), stage[:])

            def out_dma_part(p, stage, hh, half, engine):
                engine.dma_start(
                    dout[half * HALF:(half + 1) * HALF,
                         p * P + hh * DK:p * P + (hh + 1) * DK]
                    .rearrange("(ib q) c -> q ib c", q=P),
                    stage[:, half * 4:(half + 1) * 4, hh * DK:(hh + 1) * DK])

            # ---- startup: all transposes (scoped PSUM pool), then pair-0
            # QK projections; the engines' wait-queue bypass interleaves
            # the projection matmuls under the DMA-paced transposes ----
            with tc.tile_pool(name="tp_ps", bufs=1, space="PSUM") as tps:
                for rr in range(0, NB, 2):
                    tr_chunk(qt, natq[rr // 2], rr, "q", tps)
                qk_unit(0, 0, 0)
                qk_unit(0, 0, 1)
                # pair-1 projection quarters fill the DMA-paced stalls
                # between transpose chunks (wq0/wk0 slices cover pair 1 too)
                for rr in range(0, NB, 2):
                    tr_chunk(kt, natk[rr // 2], rr, "k", tps)
                    qk_unit_q(1, 0, rr // 2)
                qk_unit(0, 1, 0)
                qk_unit(0, 1, 1)
                # v transposes move into pair-0's Act-bound slots: nothing
                # else may delay the first score slot

            sc = ctx.enter_context(tc.tile_pool(name="sc", bufs=1,
                                                space="PSUM"))

            stages = {}

            def ensure_pair(p):
                if ("eT", p) not in cur:
                    cur[("eT", p)] = ring.tile([P, NB, 2, S], PVDT,
                                               name=f"eT{p}", tag="eT",
                                               bufs=2)
                if p not in stages:
                    stages[p] = ring.tile([P, NB, P], F32,
                                          name=f"stage{p}", tag="stage",
                                          bufs=2)

            def run(steps):
                for st in steps:
                    if st[0] == "sc":
                        ensure_pair(st[1])
                        score_slot(st[1], st[2], st[3])
                    else:
                        st[1]()

            # ---- pair 0 hand-scheduled: the v transposes (DMA-paced) and
            # pair-1's K projections ride its Act-bound slots ----
            ensure_pair(0)
            run([
                ("sc", 0, 0, 0),
                ("f", lambda: tr_chunk(vt, natv[0], 0, "v", scr, tag="scr")),
                ("sc", 0, 0, 1), ("f", lambda: qk_unit_q(1, 1, 0)),
                ("sc", 0, 0, 2),
                ("f", lambda: tr_chunk(vt, natv[1], 2, "v", scr, tag="scr")),
                ("sc", 0, 0, 3), ("f", lambda: qk_unit_q(1, 1, 1)),
                ("sc", 0, 0, 4),
                ("f", lambda: tr_chunk(vt, natv[2], 4, "v", scr, tag="scr")),
                ("sc", 0, 0, 5), ("f", lambda: qk_unit_q(1, 1, 2)),
                ("sc", 0, 0, 6),
                ("f", lambda: tr_chunk(vt, natv[3], 6, "v", scr, tag="scr")),
                ("sc", 0, 0, 7), ("f", lambda: qk_unit_q(1, 1, 3)),
                ("sc", 0, 1, 0), ("f", lambda: v_unit_q(0, 0)),
                ("sc", 0, 1, 1), ("f", lambda: v_unit_q(0, 1)),
                ("sc", 0, 1, 2), ("f", lambda: v_unit_q(0, 2)),
                ("sc", 0, 1, 3), ("f", lambda: v_unit_q(0, 3)),
                ("sc", 0, 1, 4), ("f", lambda: pv_unit(0, 0, 0, stages[0])),
                ("sc", 0, 1, 5), ("f", lambda: pv_unit(0, 0, 1, stages[0])),
                ("sc", 0, 1, 6), ("f", lambda: v_unit_q(1, 0)),
                ("sc", 0, 1, 7), ("f", lambda: v_unit_q(1, 1)),
            ])
            for k in (("QT", 0), ("KT", 0)):
                cur.pop(k, None)

            # ---- pairs 1..4: the original per-pair software pipeline ----
            for p in range(1, NPAIR - 3):
                ensure_pair(p)
                nxt = p + 1
                fillers = {}
                # slot-0/1 fills must not depend on the previous stretch's
                # last exp (junction stall): pv(p-1,1) sits a few slots in
                def _pv1a():
                    pv_unit(p - 1, 1, 0, stages[p - 1])
                def _pv1b():
                    pv_unit(p - 1, 1, 1, stages[p - 1])
                    out_dma(p - 1, stages[p - 1])
                if p == 1:
                    fillers[0] = lambda: v_unit_q(1, 2)
                    fillers[1] = lambda: v_unit_q(1, 3)
                    fillers[3] = _pv1a
                    fillers[5] = _pv1b
                    qslots = (2, 4, 6, 8)
                    fillers[qslots[0]] = lambda: qk_unit(nxt, 0, 0)
                    fillers[qslots[1]] = lambda: qk_unit(nxt, 0, 1)
                    fillers[qslots[2]] = lambda: qk_unit(nxt, 1, 0)
                    fillers[qslots[3]] = lambda: qk_unit(nxt, 1, 1)
                    fillers[10] = lambda: v_unit(nxt, 0)
                    fillers[14] = lambda: v_unit(nxt, 1)
                else:
                    # quarter-granular fills, one per slot; pv(p-1,1) at 6/7
                    # so the VA-ring WAR clears before the v quarters at 10+
                    for si_, qi in ((0, 0), (1, 1), (2, 2), (3, 3), (4, 4),
                                    (5, 5), (8, 6), (9, 7)):
                        fillers[si_] = (lambda qi=qi:
                                        qk_unit_q(nxt, qi // 4, qi % 4))
                    fillers[6] = _pv1a
                    fillers[7] = _pv1b
                    fillers[10] = lambda: v_unit_q(nxt, 0)
                    fillers[11] = lambda: v_unit_q(nxt, 1)
                    fillers[14] = lambda: v_unit_q(nxt, 2)
                    fillers[15] = lambda: v_unit_q(nxt, 3)
                fillers[12] = lambda: pv_unit(p, 0, 0, stages[p])
                fillers[13] = lambda: pv_unit(p, 0, 1, stages[p])
                si = 0
                for hh in range(2):
                    for jb in range(NB):
                        score_slot(p, hh, jb)
                        if si in fillers:
                            fillers[si]()
                        si += 1
                for k in (("QT", p), ("KT", p)):
                    cur.pop(k, None)

            # ---- pairs 5..7: hand-scheduled with run-ahead score slots.
            # Act (exp) needs 16.7us per 16 slots but the last pairs have
            # too little PE filler work, so pair-6 h0 scores start inside
            # pair-5's stretch and pair-7 h0 scores inside pair-6's: the
            # Act-bound final phase shrinks by the borrowed slots. ----
            ensure_pair(5)
            run([
                ("sc", 5, 0, 0), ("f", lambda: qk_unit_q(6, 0, 0)),
                ("sc", 5, 0, 1), ("f", lambda: qk_unit_q(6, 0, 1)),
                ("sc", 5, 0, 2), ("f", lambda: qk_unit_q(6, 0, 2)),
                ("sc", 5, 0, 3), ("f", lambda: qk_unit_q(6, 0, 3)),
                ("sc", 5, 0, 4), ("f", lambda: qk_unit_q(6, 1, 0)),
                ("sc", 5, 0, 5), ("f", lambda: qk_unit_q(6, 1, 1)),
                ("sc", 5, 0, 6), ("f", lambda: qk_unit_q(6, 1, 2)),
                ("sc", 5, 0, 7), ("f", lambda: qk_unit_q(6, 1, 3)),
                ("sc", 5, 1, 0), ("f", lambda: pv_unit(4, 1, 0, stages[4])),
                ("sc", 5, 1, 1),
                ("f", lambda: (pv_unit(4, 1, 1, stages[4]),
                               out_dma(4, stages[4]))),
                ("sc", 5, 1, 2), ("f", lambda: v_unit_q(6, 0)),
                ("sc", 6, 0, 0),
                ("sc", 5, 1, 3), ("f", lambda: v_unit_q(6, 1)),
                ("sc", 6, 0, 1),
                ("sc", 5, 1, 4), ("f", lambda: pv_unit(5, 0, 0, stages[5])),
                ("sc", 6, 0, 2),
                ("sc", 5, 1, 5), ("f", lambda: pv_unit(5, 0, 1, stages[5])),
                ("sc", 6, 0, 3),
                ("sc", 5, 1, 6), ("f", lambda: v_unit_q(6, 2)),
                ("sc", 5, 1, 7), ("f", lambda: v_unit_q(6, 3)),
            ])
            for k in (("QT", 5), ("KT", 5)):
                cur.pop(k, None)

            # pair-6 stretch absorbs ALL of pair-7's h0 scores and the first
            # two h1 scores: Act saturates from here to the end while PE
            # (balanced ~23us work vs ~22.8us Act) keeps feeding it
            run([
                ("sc", 6, 0, 4), ("f", lambda: qk_unit_q(7, 0, 0)),
                ("sc", 6, 0, 5), ("f", lambda: qk_unit_q(7, 0, 1)),
                ("sc", 6, 0, 6), ("f", lambda: qk_unit_q(7, 0, 2)),
                ("sc", 6, 0, 7), ("f", lambda: qk_unit_q(7, 0, 3)),
                ("sc", 6, 1, 0), ("f", lambda: qk_unit_q(7, 1, 0)),
                ("sc", 6, 1, 1), ("f", lambda: qk_unit_q(7, 1, 1)),
                ("sc", 6, 1, 2), ("f", lambda: qk_unit_q(7, 1, 2)),
                ("sc", 6, 1, 3), ("f", lambda: qk_unit_q(7, 1, 3)),
                ("sc", 6, 1, 4), ("f", lambda: pv_unit(5, 1, 0, stages[5])),
                ("sc", 6, 1, 5),
                ("f", lambda: (pv_unit(5, 1, 1, stages[5]),
                               out_dma(5, stages[5]))),
                ("sc", 6, 1, 6),
                ("sc", 7, 0, 0), ("f", lambda: pv_unit(6, 0, 0, stages[6])),
                ("sc", 6, 1, 7),
                ("sc", 7, 0, 1), ("f", lambda: pv_unit(6, 0, 1, stages[6])),
                ("sc", 7, 0, 2), ("f", lambda: v_unit_q(7, 0)),
                ("sc", 7, 0, 3),
                ("sc", 7, 0, 4), ("f", lambda: v_unit_q(7, 1)),
                ("sc", 7, 0, 5),
                ("sc", 7, 0, 6), ("f", lambda: pv_unit(6, 1, 0, stages[6])),
                ("sc", 7, 0, 7),
                ("sc", 7, 1, 0),
                ("f", lambda: (pv_unit(6, 1, 1, stages[6]),
                               out_dma(6, stages[6]))),
                ("sc", 7, 1, 1),
            ])
            for k in (("QT", 6), ("KT", 6)):
                cur.pop(k, None)

            # final stretch: the last six scores issue with nothing heavy in
            # front of them; all deferrable PV/V work runs AFTER the last
            # score, overlapping the final exps
            run([
                ("sc", 7, 1, 2),
                ("sc", 7, 1, 3), ("f", lambda: v_unit_q(7, 2)),
                ("sc", 7, 1, 4), ("f", lambda: v_unit_q(7, 3)),
                ("sc", 7, 1, 5), ("f", lambda: pv_unit(7, 0, 0, stages[7])),
                ("sc", 7, 1, 6),
                ("f", lambda: (pv_unit(7, 0, 1, stages[7]),
                               out_dma_part(7, stages[7], 0, 0, nc.sync),
                               out_dma_part(7, stages[7], 0, 1, nc.sync))),
                ("sc", 7, 1, 7),
            ])
            for k in (("QT", 7), ("KT", 7)):
                cur.pop(k, None)

            # tail: final head's PV; h0 normalizes on DVE, h1 on the (now
            # idle) ScalarE so the two norm chains overlap; DMAs on
            # different queues
            stage = stages[NPAIR - 1]
            eTp7, VAp7 = cur[("eT", NPAIR - 1)], cur[("VA", NPAIR - 1)]
            pus = []
            for half in range(2):
                pu = scr.tile([P, HALF], F32, name=f"pu_t_{half}",
                              tag="scr", bufs=2)
                pu4 = pu.rearrange("q (i f) -> q i f", f=P)
                pv_mms(pu4, eTp7, VAp7, 1, half, list(range(NB)),
                       True, True)
                pus.append(pu4)
            for half in range(2):
                pv_norm(pus[half], stage, 1, half)
                out_dma_part(NPAIR - 1, stage, 1, half,
                             nc.scalar if half == 0 else nc.sync)

    nc.compile()
    return nc


def kernel(q, k, v, w_q, b_q, w_k, b_k, w_v, b_v):
    use_bias = bool(np.any(np.asarray(b_q)) or np.any(np.asarray(b_k))
                    or np.any(np.asarray(b_v)))
    key = (use_bias, PV_FP8, _NO_DR)
    if key not in _compiled:
        _compiled[key] = _build(use_bias, PV_FP8)
    nc = _compiled[key]

    f = lambda x: np.ascontiguousarray(np.asarray(x, dtype=np.float32))
    in_maps = []
    for c in range(N_CORES):
        in_maps.append({
            "q": f(q[c]), "k": f(k[c]), "v": f(v[c]),
            "wq": f(w_q), "wk": f(w_k), "wv": f(w_v),
            "bq": f(b_q), "bk": f(b_k), "bv": f(b_v),
        })
    res = run_bass_kernel_spmd(nc, in_maps, list(range(N_CORES)))
    out = np.stack([res.results[c]["out"] for c in range(N_CORES)], axis=0)
    kernel.last_results = res
    return out

